# revision 3
# baseline (speedup 1.0000x reference)
"""Trainium2 Bass kernel for nn_CLUBCategorical (CLUB categorical loss).

Reference computation:
    h      = relu(x @ W1 + b1)              [N, H]
    logits = h @ W2 + b2                    [N, Y]
    logp   = log_softmax(logits, -1)        [N, Y]
    out[i] = logp[i, y_i] - mean_j logp[i, y_j]

Algebraic simplification used here: with c[y] = histogram(y_idx) the
log-softmax normalizer cancels between the positive and negative terms:

    out[i] = L[i, y_i] - (1/N) * (L[i, :] @ c) + (b2[y_i] - (b2 @ c)/N)

where L = relu(x @ W1 + b1) @ W2 (no bias, no softmax). On device this is
two dense matmuls plus a masked column reduction:

    out[i] = sum_y L[i, y] * (onehot(y_i)[y] - c[y]/N) + g[i]

Sharding: data-parallel over N. Each of the 8 cores handles 1024 rows and
gets the full W1/W2 plus the global label histogram (the "all-gather of
column labels" is precomputed on host into c). No collectives needed.

Device layout (per core, everything transposed so the contraction dim is
on SBUF partitions):
    phase 1: hT[j]   [128h, 1024r] = W1[k,jslice].T @ xT[k, rows]  (+b1, relu)
    phase 2: psum_l  [128y,  512r] = W2[j,qslice].T @ hT[j, rows]
             eqc     [128y,  512r] = (ybc == iota_q) - cN_q        (DVE)
             prod    = psum_l * eqc                                (DVE)
             out     += ones.T @ prod  (M=1 matmul, reduce over y) (PE)
Matmuls run in float32r (~TF32 precision, 4x faster than fp32 on PE).
"""

import numpy as np

N, X_DIM, Y_DIM, HIDDEN = 8192, 512, 512, 1024
N_CORES = 8
N_LOC = N // N_CORES          # 1024 rows per core
KX = X_DIM // 128             # 4  k-chunks, phase 1
KH = HIDDEN // 128            # 8  k-chunks, phase 2 / m-chunks, phase 1
QY = Y_DIM // 128             # 4  y-chunks, phase 2
RG = N_LOC // 512             # 2  row groups of 512

_NC_CACHE = {}


def _build(nc_cls, mybir, tile):
    mdt = mybir.dt
    f32 = mdt.float32
    F32R = mdt.float32r
    AF = mybir.ActivationFunctionType
    OP = mybir.AluOpType

    nc = nc_cls("TRN2", target_bir_lowering=False, debug=False,
                num_devices=N_CORES)

    xT = nc.dram_tensor("xT", [X_DIM, N_LOC], f32, kind="ExternalInput")
    W1 = nc.dram_tensor("W1", [X_DIM, HIDDEN], f32, kind="ExternalInput")
    W2 = nc.dram_tensor("W2", [HIDDEN, Y_DIM], f32, kind="ExternalInput")
    b1c = nc.dram_tensor("b1c", [128, KH], f32, kind="ExternalInput")
    ybc = nc.dram_tensor("ybc", [128, N_LOC], f32, kind="ExternalInput")
    iot = nc.dram_tensor("iot", [128, QY], f32, kind="ExternalInput")
    cNc = nc.dram_tensor("cNc", [128, QY], f32, kind="ExternalInput")
    onesv = nc.dram_tensor("onesv", [128, 1], f32, kind="ExternalInput")
    gv = nc.dram_tensor("gv", [1, N_LOC], f32, kind="ExternalInput")
    out = nc.dram_tensor("out", [1, N_LOC], f32, kind="ExternalOutput")

    with tile.TileContext(nc) as tc:
        with (
            tc.tile_pool(name="wgt", bufs=1) as wgt,
            tc.tile_pool(name="hp", bufs=1) as hp,
            tc.tile_pool(name="eqp", bufs=1) as eqp,
            tc.tile_pool(name="prp", bufs=4) as prp,
            tc.tile_pool(name="osb", bufs=1) as osb,
            tc.tile_pool(name="ph", bufs=3, space="PSUM") as ph,
            tc.tile_pool(name="pl", bufs=3, space="PSUM") as pl,
            tc.tile_pool(name="po", bufs=1, space="PSUM") as po,
        ):
            # DMA issue is serialized per DGE queue, so spread the loads
            # across all three queues: sync HWDGE (xT path), scalar HWDGE
            # (W1 path, issued before any ACTIVATE), gpsimd SWDGE (W2 +
            # small constants, needed later).
            b1_sb = wgt.tile([128, KH], f32, tag="b1")
            nc.gpsimd.dma_start(b1_sb[:], b1c.ap())
            # phase-1 operands, k-interleaved so MMs start early
            w1_sb, xt_sb = [], []
            for k in range(KX):
                xtt = wgt.tile([128, N_LOC], F32R, tag=f"xt_{k}")
                nc.sync.dma_start(
                    xtt[:], xT.ap()[k * 128:(k + 1) * 128, :].bitcast(F32R))
                xt_sb.append(xtt)
                w1t = wgt.tile([128, HIDDEN], F32R, tag=f"w1_{k}")
                nc.scalar.dma_start(
                    w1t[:], W1.ap()[k * 128:(k + 1) * 128, :].bitcast(F32R))
                w1_sb.append(w1t)
            # phase-2 weights + masks (needed ~15us in)
            w2_sb = []
            for j in range(KH):
                w2t = wgt.tile([128, Y_DIM], F32R, tag=f"w2_{j}")
                nc.gpsimd.dma_start(
                    w2t[:], W2.ap()[j * 128:(j + 1) * 128, :].bitcast(F32R))
                w2_sb.append(w2t)
            ybc_sb = wgt.tile([128, N_LOC], f32, tag="ybc")
            nc.sync.dma_start(ybc_sb[:], ybc.ap())
            iot_sb = wgt.tile([128, QY], f32, tag="iot")
            nc.gpsimd.dma_start(iot_sb[:], iot.ap())
            cnc_sb = wgt.tile([128, QY], f32, tag="cnc")
            nc.gpsimd.dma_start(cnc_sb[:], cNc.ap())
            ones_sb = wgt.tile([128, 1], F32R, tag="ones")
            nc.gpsimd.dma_start(ones_sb[:], onesv.ap().bitcast(F32R))
            g_sb = wgt.tile([1, N_LOC], f32, tag="g")
            nc.gpsimd.dma_start(g_sb[:], gv.ap())

            # --- eqc masks (DVE; no matmul dependency, fills DVE idle time)
            eqc_sb = {}
            for n in range(RG):
                for q in range(QY):
                    e = eqp.tile([128, 512], f32, tag=f"eqc_{n}_{q}")
                    nc.vector.tensor_scalar(
                        e[:], ybc_sb[:, n * 512:(n + 1) * 512],
                        iot_sb[:, q:q + 1], cnc_sb[:, q:q + 1],
                        OP.is_equal, OP.subtract)
                    eqc_sb[(n, q)] = e

            # --- phase 1: hT[j] = relu(W1.T @ xT + b1) ---
            hT = [hp.tile([128, N_LOC], F32R, tag=f"h_{j}", name=f"h_{j}")
                  for j in range(KH)]
            for m in range(KH):
                for n in range(RG):
                    psum = ph.tile([128, 512], f32)
                    for k in range(KX):
                        nc.tensor.matmul(
                            psum[:],
                            w1_sb[k][:, m * 128:(m + 1) * 128],
                            xt_sb[k][:, n * 512:(n + 1) * 512],
                            start=(k == 0), stop=(k == KX - 1))
                    nc.scalar.activation(
                        hT[m][:, n * 512:(n + 1) * 512], psum[:],
                        AF.Relu, bias=b1_sb[:, m:m + 1])

            # --- phase 2 ---
            pout = {n: po.tile([1, 512], f32, tag=f"po_{n}", name=f"po_{n}")
                    for n in range(RG)}
            pending = []  # delay ones-MMs one (n,q) step so PE never waits on DVE

            def flush_one():
                n, q, prod = pending.pop(0)
                nc.tensor.matmul(
                    pout[n][:], ones_sb[:], prod[:],
                    start=(q == 0), stop=(q == QY - 1))

            for n in range(RG):
                for q in range(QY):
                    psum_l = pl.tile([128, 512], f32)
                    for j in range(KH):
                        nc.tensor.matmul(
                            psum_l[:],
                            w2_sb[j][:, q * 128:(q + 1) * 128],
                            hT[j][:, n * 512:(n + 1) * 512],
                            start=(j == 0), stop=(j == KH - 1))
                    prod = prp.tile([128, 512], F32R)
                    nc.vector.tensor_tensor(
                        prod[:], psum_l[:], eqc_sb[(n, q)][:], OP.mult)
                    pending.append((n, q, prod))
                    if len(pending) >= 2:
                        flush_one()
            while pending:
                flush_one()

            # --- epilogue: add g, store ---
            for n in range(RG):
                o = osb.tile([1, 512], f32, tag=f"o_{n}")
                nc.vector.tensor_tensor(
                    o[:], pout[n][:], g_sb[:, n * 512:(n + 1) * 512], OP.add)
                nc.sync.dma_start(out.ap()[0:1, n * 512:(n + 1) * 512], o[:])

    nc.compile()
    return nc


def _get_nc():
    if "nc" not in _NC_CACHE:
        import concourse.bacc as bacc
        import concourse.mybir as mybir
        from concourse import tile
        _NC_CACHE["nc"] = _build(bacc.Bacc, mybir, tile)
    return _NC_CACHE["nc"]


def kernel(x_samples, y_idx, W1, b1, W2, b2):
    from concourse.bass_utils import run_bass_kernel_spmd

    x = np.ascontiguousarray(np.asarray(x_samples, dtype=np.float32))
    y = np.asarray(y_idx).astype(np.int64).reshape(-1)
    W1 = np.ascontiguousarray(np.asarray(W1, dtype=np.float32))
    b1 = np.asarray(b1, dtype=np.float32).reshape(-1)
    W2 = np.ascontiguousarray(np.asarray(W2, dtype=np.float32))
    b2 = np.asarray(b2, dtype=np.float32).reshape(-1)

    # global label histogram + fully-folded bias term
    c = np.bincount(y, minlength=Y_DIM).astype(np.float32)
    cN = c / np.float32(N)
    beta = np.float32(b2 @ c) / np.float32(N)
    g_full = (b2[y] - beta).astype(np.float32)

    xT = np.ascontiguousarray(x.T)                                # [512, 8192]
    b1c = np.ascontiguousarray(b1.reshape(KH, 128).T)             # [128, 8]
    iot = np.ascontiguousarray(
        np.arange(Y_DIM, dtype=np.float32).reshape(QY, 128).T)    # [128, 4]
    cNc = np.ascontiguousarray(cN.reshape(QY, 128).T)             # [128, 4]
    onesv = np.ones((128, 1), dtype=np.float32)

    in_maps = []
    for m in range(N_CORES):
        sl = slice(m * N_LOC, (m + 1) * N_LOC)
        y_loc = y[sl].astype(np.float32)
        in_maps.append({
            "xT": np.ascontiguousarray(xT[:, sl]),
            "W1": W1,
            "W2": W2,
            "b1c": b1c,
            "ybc": np.ascontiguousarray(
                np.broadcast_to(y_loc[None, :], (128, N_LOC))),
            "iot": iot,
            "cNc": cNc,
            "onesv": onesv,
            "gv": np.ascontiguousarray(g_full[sl]).reshape(1, N_LOC),
        })

    nc = _get_nc()
    res = run_bass_kernel_spmd(nc, in_maps, core_ids=list(range(N_CORES)))
    return np.concatenate(
        [res.results[m]["out"].reshape(-1) for m in range(N_CORES)]
    ).astype(np.float32)


# revision 4
# speedup vs baseline: 1.0964x; 1.0964x over previous
"""Trainium2 Bass kernel for nn_CLUBCategorical (CLUB categorical loss).

Reference computation:
    h      = relu(x @ W1 + b1)              [N, H]
    logits = h @ W2 + b2                    [N, Y]
    logp   = log_softmax(logits, -1)        [N, Y]
    out[i] = logp[i, y_i] - mean_j logp[i, y_j]

Algebraic simplification used here: with c[y] = histogram(y_idx) the
log-softmax normalizer cancels between the positive and negative terms:

    out[i] = L[i, y_i] - (1/N) * (L[i, :] @ c) + (b2[y_i] - (b2 @ c)/N)

where L = relu(x @ W1 + b1) @ W2 (no bias, no softmax). On device this is
two dense matmuls plus a masked column reduction:

    out[i] = sum_y L[i, y] * (onehot(y_i)[y] - c[y]/N) + g[i]

Sharding: data-parallel over N. Each of the 8 cores handles 1024 rows and
gets the full W1/W2 plus the global label histogram (the "all-gather of
column labels" is precomputed on host into c). No collectives needed.

Device layout (per core, everything transposed so the contraction dim is
on SBUF partitions):
    phase 1: hT[j]   [128h, 1024r] = W1[k,jslice].T @ xT[k, rows]  (+b1, relu)
    phase 2: psum_l  [128y,  512r] = W2[j,qslice].T @ hT[j, rows]
             eqc     [128y,  512r] = (ybc == iota_q) - cN_q        (DVE)
             prod    = psum_l * eqc                                (DVE)
             out     += ones.T @ prod  (M=1 matmul, reduce over y) (PE)
Matmuls run in float32r (~TF32 precision, 4x faster than fp32 on PE).
"""

import numpy as np

N, X_DIM, Y_DIM, HIDDEN = 8192, 512, 512, 1024
N_CORES = 8
N_LOC = N // N_CORES          # 1024 rows per core
KX = X_DIM // 128             # 4  k-chunks, phase 1
KH = HIDDEN // 128            # 8  k-chunks, phase 2 / m-chunks, phase 1
QY = Y_DIM // 128             # 4  y-chunks, phase 2
RG = N_LOC // 512             # 2  row groups of 512

_NC_CACHE = {}


def _build(nc_cls, mybir, tile):
    mdt = mybir.dt
    f32 = mdt.float32
    F32R = mdt.float32r
    AF = mybir.ActivationFunctionType
    OP = mybir.AluOpType

    nc = nc_cls("TRN2", target_bir_lowering=False, debug=False,
                num_devices=N_CORES)

    xT = nc.dram_tensor("xT", [X_DIM, N_LOC], f32, kind="ExternalInput")
    W1 = nc.dram_tensor("W1", [X_DIM, HIDDEN], f32, kind="ExternalInput")
    W2 = nc.dram_tensor("W2", [HIDDEN, Y_DIM], f32, kind="ExternalInput")
    # packed constants: [b1c(8) | iot(4) | cNc(4) | ones(1)] = [128, 17]
    cst = nc.dram_tensor("cst", [128, KH + 2 * QY + 1], f32,
                         kind="ExternalInput")
    ybc = nc.dram_tensor("ybc", [128, N_LOC], f32, kind="ExternalInput")
    gv = nc.dram_tensor("gv", [1, N_LOC], f32, kind="ExternalInput")
    out = nc.dram_tensor("out", [1, N_LOC], f32, kind="ExternalOutput")

    with tile.TileContext(nc) as tc:
        with (
            tc.tile_pool(name="wgt", bufs=1) as wgt,
            tc.tile_pool(name="hp", bufs=1) as hp,
            tc.tile_pool(name="eqp", bufs=1) as eqp,
            tc.tile_pool(name="prp", bufs=4) as prp,
            tc.tile_pool(name="osb", bufs=1) as osb,
            tc.tile_pool(name="ph", bufs=3, space="PSUM") as ph,
            tc.tile_pool(name="pl", bufs=3, space="PSUM") as pl,
            tc.tile_pool(name="po", bufs=1, space="PSUM") as po,
        ):
            # DMA issue is serialized per DGE queue. Priority order:
            # phase-1 operands first (xT on sync HWDGE, W1 on scalar
            # HWDGE, interleaved by k so MMs start early); W2 follows W1
            # on the scalar queue (needed only when phase 2 starts);
            # small constants on the gpsimd SWDGE queue.
            cst_sb = wgt.tile([128, KH + 2 * QY + 1], F32R, tag="cst")
            nc.gpsimd.dma_start(cst_sb[:], cst.ap().bitcast(F32R))
            b1_sb = cst_sb[:, 0:KH].bitcast(f32)
            iot_sb = cst_sb[:, KH:KH + QY].bitcast(f32)
            cnc_sb = cst_sb[:, KH + QY:KH + 2 * QY].bitcast(f32)
            ones_sb = cst_sb[:, KH + 2 * QY:KH + 2 * QY + 1]
            g_sb = wgt.tile([1, N_LOC], f32, tag="g")
            nc.gpsimd.dma_start(g_sb[:], gv.ap())
            w1_sb, xt_sb = [], []
            for k in range(KX):
                xtt = wgt.tile([128, N_LOC], F32R, tag=f"xt_{k}")
                nc.sync.dma_start(
                    xtt[:], xT.ap()[k * 128:(k + 1) * 128, :].bitcast(F32R))
                xt_sb.append(xtt)
                w1t = wgt.tile([128, HIDDEN], F32R, tag=f"w1_{k}")
                nc.scalar.dma_start(
                    w1t[:], W1.ap()[k * 128:(k + 1) * 128, :].bitcast(F32R))
                w1_sb.append(w1t)
            ybc_sb = wgt.tile([128, N_LOC], f32, tag="ybc")
            nc.sync.dma_start(ybc_sb[:], ybc.ap())
            # W2 as two 1MB descriptors after W1 on the scalar queue
            w2p_sb = []
            for h in range(2):
                w2p = wgt.tile([128, 4 * Y_DIM], F32R, tag=f"w2p_{h}",
                               name=f"w2p_{h}")
                nc.scalar.dma_start(
                    w2p[:].rearrange("p (a y) -> p a y", a=4),
                    W2.ap()[h * 512:(h + 1) * 512, :]
                    .rearrange("(a p) y -> p a y", p=128).bitcast(F32R))
                w2p_sb.append(w2p)
            w2_sb = [w2p_sb[j // 4][:, (j % 4) * Y_DIM:(j % 4 + 1) * Y_DIM]
                     for j in range(KH)]

            # --- eqc masks (DVE; no matmul dependency, fills DVE idle time)
            eqc_sb = {}
            for n in range(RG):
                for q in range(QY):
                    e = eqp.tile([128, 512], f32, tag=f"eqc_{n}_{q}")
                    nc.vector.tensor_scalar(
                        e[:], ybc_sb[:, n * 512:(n + 1) * 512],
                        iot_sb[:, q:q + 1], cnc_sb[:, q:q + 1],
                        OP.is_equal, OP.subtract)
                    eqc_sb[(n, q)] = e

            # --- phase 1: hT[j] = relu(W1.T @ xT + b1) ---
            hT = [hp.tile([128, N_LOC], F32R, tag=f"h_{j}", name=f"h_{j}")
                  for j in range(KH)]
            for m in range(KH):
                for n in range(RG):
                    psum = ph.tile([128, 512], f32)
                    for k in range(KX):
                        nc.tensor.matmul(
                            psum[:],
                            w1_sb[k][:, m * 128:(m + 1) * 128],
                            xt_sb[k][:, n * 512:(n + 1) * 512],
                            start=(k == 0), stop=(k == KX - 1))
                    nc.scalar.activation(
                        hT[m][:, n * 512:(n + 1) * 512], psum[:],
                        AF.Relu, bias=b1_sb[:, m:m + 1])

            # --- phase 2 ---
            pout = {n: po.tile([1, 512], f32, tag=f"po_{n}", name=f"po_{n}")
                    for n in range(RG)}
            pending = []  # delay ones-MMs one (n,q) step so PE never waits on DVE

            def flush_one():
                n, q, prod = pending.pop(0)
                nc.tensor.matmul(
                    pout[n][:], ones_sb, prod[:],
                    start=(q == 0), stop=(q == QY - 1))

            for n in range(RG):
                for q in range(QY):
                    psum_l = pl.tile([128, 512], f32)
                    for j in range(KH):
                        nc.tensor.matmul(
                            psum_l[:],
                            w2_sb[j][:, q * 128:(q + 1) * 128],
                            hT[j][:, n * 512:(n + 1) * 512],
                            start=(j == 0), stop=(j == KH - 1))
                    prod = prp.tile([128, 512], F32R)
                    nc.vector.tensor_tensor(
                        prod[:], psum_l[:], eqc_sb[(n, q)][:], OP.mult)
                    pending.append((n, q, prod))
                    if len(pending) >= 2:
                        flush_one()
            while pending:
                flush_one()

            # --- epilogue: add g, store (single output DMA) ---
            o = osb.tile([1, N_LOC], f32, tag="o")
            for n in range(RG):
                nc.vector.tensor_tensor(
                    o[:, n * 512:(n + 1) * 512], pout[n][:],
                    g_sb[:, n * 512:(n + 1) * 512], OP.add)
            nc.sync.dma_start(out.ap(), o[:])

    nc.compile()
    return nc


def _get_nc():
    if "nc" not in _NC_CACHE:
        import concourse.bacc as bacc
        import concourse.mybir as mybir
        from concourse import tile
        _NC_CACHE["nc"] = _build(bacc.Bacc, mybir, tile)
    return _NC_CACHE["nc"]


def kernel(x_samples, y_idx, W1, b1, W2, b2):
    from concourse.bass_utils import run_bass_kernel_spmd

    x = np.ascontiguousarray(np.asarray(x_samples, dtype=np.float32))
    y = np.asarray(y_idx).astype(np.int64).reshape(-1)
    W1 = np.ascontiguousarray(np.asarray(W1, dtype=np.float32))
    b1 = np.asarray(b1, dtype=np.float32).reshape(-1)
    W2 = np.ascontiguousarray(np.asarray(W2, dtype=np.float32))
    b2 = np.asarray(b2, dtype=np.float32).reshape(-1)

    # global label histogram + fully-folded bias term
    c = np.bincount(y, minlength=Y_DIM).astype(np.float32)
    cN = c / np.float32(N)
    beta = np.float32(b2 @ c) / np.float32(N)
    g_full = (b2[y] - beta).astype(np.float32)

    xT = np.ascontiguousarray(x.T)                                # [512, 8192]
    b1c = b1.reshape(KH, 128).T                                   # [128, 8]
    iot = np.arange(Y_DIM, dtype=np.float32).reshape(QY, 128).T   # [128, 4]
    cNc = cN.reshape(QY, 128).T                                   # [128, 4]
    onesv = np.ones((128, 1), dtype=np.float32)
    cst = np.ascontiguousarray(
        np.concatenate([b1c, iot, cNc, onesv], axis=1))           # [128, 17]

    in_maps = []
    for m in range(N_CORES):
        sl = slice(m * N_LOC, (m + 1) * N_LOC)
        y_loc = y[sl].astype(np.float32)
        in_maps.append({
            "xT": np.ascontiguousarray(xT[:, sl]),
            "W1": W1,
            "W2": W2,
            "cst": cst,
            "ybc": np.ascontiguousarray(
                np.broadcast_to(y_loc[None, :], (128, N_LOC))),
            "gv": np.ascontiguousarray(g_full[sl]).reshape(1, N_LOC),
        })

    nc = _get_nc()
    res = run_bass_kernel_spmd(nc, in_maps, core_ids=list(range(N_CORES)))
    return np.concatenate(
        [res.results[m]["out"].reshape(-1) for m in range(N_CORES)]
    ).astype(np.float32)
